# revision 1
# baseline (speedup 1.0000x reference)
"""Bidirectional Mamba block (BiT_MamSleep) on 8 TRN2 NeuronCores — v6.

Sharding: core c handles (batch b = c//2, direction dir = c%2); pairwise
AllReduce joins the two directions; both cores compute the tail redundantly.

s-major scan layout: 32 tiles of [128 part = d (one half of d_inner),
free = t], one per (half h, state s).  dA_s comes straight from ACT exp with
per-partition scale A[:, s]; B/C are row-broadcast per state; the sum over s
is identity-matmul PSUM accumulation on PE.  The depthwise conv is folded
into the in-projection (4 shifted-AP matmuls).  All matmuls bf16.

All elementwise work runs on DVE: GpSimd tensor ops contend with DVE for the
shared SBUF port and slow concurrent DVE instructions by ~80% (measured), so
GpSimd only drives the collectives.  The direction flip + select is folded
into the out-projection via per-core masked forward/reversed weights.  The
exchange is split per half h: the h=0 AllReduce runs hidden under the h=1
scan block.
"""
import sys

if '/opt/trn_rl_repo' not in sys.path:
    sys.path.insert(0, '/opt/trn_rl_repo')

import ml_dtypes
import numpy as np

import concourse.bass as bass
import concourse.bacc as bacc
import concourse.tile as tile
from concourse import mybir
from concourse.bass_utils import run_bass_kernel_spmd

HID = 128
BATCH = 4
SEQ = 2048
D_STATE = 16
D_CONV = 4
D_INNER = 256
DT_RANK = 8

L = SEQ
C = HID
CW = 512
NCH = L // CW
PW = 1024
NPW = L // PW
f32 = mybir.dt.float32
bf16 = mybir.dt.bfloat16
mult = mybir.AluOpType.mult
add = mybir.AluOpType.add
sub = mybir.AluOpType.subtract
AF = mybir.ActivationFunctionType

_PROGRAM = None


def _declare(nc):
    dpf = lambda name, shape: nc.declare_dram_parameter(name, list(shape), f32,
                                                        isOutput=False)
    dph = lambda name, shape: nc.declare_dram_parameter(name, list(shape), bf16,
                                                        isOutput=False)
    p = {}
    p['x'] = dpf('x', (C, L))
    for n in ('wlmT', 'wlgT', 'wcT', 'loT'):
        p[n] = dph(n, (C, C))
    for h in range(2):
        for k in range(D_CONV):
            p[f'wk{h}{k}'] = dph(f'wk{h}{k}', (C, C))
        p[f'inwzT{h}'] = dph(f'inwzT{h}', (C, C))
        p[f'owTA{h}'] = dph(f'owTA{h}', (128, C))
        p[f'owTB{h}'] = dph(f'owTB{h}', (128, C))
        p[f'xpwT{h}'] = dph(f'xpwT{h}', (128, DT_RANK + 2 * D_STATE))
        p[f'dtwT{h}'] = dph(f'dtwT{h}', (DT_RANK, 128))
    p['ident'] = dph('ident', (128, 128))
    p['avec'] = dpf('avec', (128, 32))
    for n in ('conv_b', 'dt_b', 'dp_v'):
        p[n] = dpf(n, (128, 2))
    for n in ('bias_lm', 'bias_lg', 'bias_c', 'lo_b', 'ln_g', 'ln_b'):
        p[n] = dpf(n, (C, 1))
    p['y'] = nc.declare_dram_parameter('y', [C, L], f32, isOutput=True)
    return p


class B:
    pass


def _ln_stats_mm(b, x_sb, ones_in, out_bf):
    """LayerNorm over the 128 channels per column; stage-major to keep the
    ACT table set stable."""
    nc = b.nc
    rows = b.pf.tile([128, L], f32, name='lnrows', tag='f')
    rows_bf = b.pb.tile([2, L], bf16, name='lnrowsb', tag='lnb')
    ex = rows_bf[0:1, :]
    nrm0 = b.pb.tile([C, L], bf16, name='nrm0', tag='xmf')
    sq2 = b.pb.tile([C, L], bf16, name='sq2', tag='y0')
    rr = rows[0:1, :]
    csl = [slice(ci * CW, (ci + 1) * CW) for ci in range(NCH)]
    for cs in csl:
        ps0 = b.ps.tile([1, CW], f32, name='bank', tag='bank')
        nc.tensor.matmul(ps0, ones_in, x_sb[:, cs], start=True, stop=True)
        nc.scalar.activation(ex[:, cs], ps0, AF.Identity, bias=0.0, scale=1.0 / C)
    for cs in csl:
        psb = b.ps.tile([128, CW], f32, name='bank', tag='bank')
        nc.tensor.matmul(psb, b.ones_row, ex[:, cs], start=True, stop=True)
        nc.vector.scalar_tensor_tensor(nrm0[:, cs], x_sb[:, cs], 1.0, psb,
                                       mult, sub)
    for cs in csl:
        nc.scalar.activation(sq2[:, cs], nrm0[:, cs], AF.Square)
    for cs in csl:
        psv = b.ps.tile([1, CW], f32, name='bank', tag='bank')
        nc.tensor.matmul(psv, b.ones_col, sq2[:, cs], start=True, stop=True)
        nc.scalar.activation(rr[:, cs], psv, AF.Ln, bias=b.eps_t[:, :],
                             scale=1.0 / C)
    for cs in csl:
        nc.scalar.activation(ex[:, cs], rr[:, cs], AF.Exp, bias=0.0, scale=-0.5)
    for cs in csl:
        psr = b.ps.tile([128, CW], f32, name='bank', tag='bank')
        nc.tensor.matmul(psr, b.ones_row, ex[:, cs], start=True, stop=True)
        nc.vector.scalar_tensor_tensor(out_bf[:, cs], nrm0[:, cs], 1.0, psr,
                                       mult, mult)


def _proj(b, lhsT, rhs, out, func, bias, rows=C, out_off=0):
    nc = b.nc
    for pi in range(NPW):
        ps = b.ps.tile([rows, PW], f32, name='bank', tag='bank')
        for half in range(2):
            cs = slice(pi * PW + half * CW, pi * PW + (half + 1) * CW)
            nc.tensor.matmul(ps[:, half * CW:(half + 1) * CW], lhsT, rhs[:, cs],
                             start=True, stop=True)
        ocs = slice(out_off + pi * PW, out_off + (pi + 1) * PW)
        nc.scalar.activation(out[:, ocs], ps, func, bias=bias)


def _build_body(nc, tc, p, ctx):
    b = B()
    b.nc = nc
    b.io = ctx.enter_context(tc.tile_pool(name='io', bufs=1))
    b.pb = ctx.enter_context(tc.tile_pool(name='pb', bufs=1))
    b.pf = ctx.enter_context(tc.tile_pool(name='pf', bufs=3))
    b.bc = ctx.enter_context(tc.tile_pool(name='bc', bufs=3))
    b.cb = ctx.enter_context(tc.tile_pool(name='cb', bufs=3))
    b.da = ctx.enter_context(tc.tile_pool(name='da', bufs=3))
    b.du = ctx.enter_context(tc.tile_pool(name='du', bufs=3))
    b.ht = ctx.enter_context(tc.tile_pool(name='ht', bufs=3))
    b.yc = ctx.enter_context(tc.tile_pool(name='yc', bufs=3))
    b.ps = ctx.enter_context(tc.tile_pool(name='ps', bufs=2, space='PSUM'))
    b.py = ctx.enter_context(tc.tile_pool(name='py', bufs=1, space='PSUM'))
    b.dram = ctx.enter_context(tc.tile_pool(name='drm', bufs=1, space='DRAM'))

    x = b.pf.tile([C, L], f32, name='x', tag='f')
    nc.sync.dma_start(out=x, in_=p['x'][:, :])

    W = {}
    wspec = [('wlmT', (C, C)), ('wlgT', (C, C)), ('wcT', (C, C)),
             ('loT', (C, C)), ('ident', (128, 128))]
    for h in range(2):
        wspec += [(f'wk{h}{k}', (C, C)) for k in range(D_CONV)]
        wspec += [(f'inwzT{h}', (C, C)), (f'owTA{h}', (128, C)),
                  (f'owTB{h}', (128, C)),
                  (f'xpwT{h}', (128, 40)), (f'dtwT{h}', (8, 128))]
    for n, shape in wspec:
        if n.startswith('dtwT'):
            W[n] = b.io.tile([40, shape[1]], bf16, name=n, tag=n)
            nc.sync.dma_start(out=W[n][32:40, :], in_=p[n][:, :])
            W[n] = W[n][32:40, :]
        else:
            W[n] = b.io.tile(list(shape), bf16, name=n, tag=n)
            nc.sync.dma_start(out=W[n], in_=p[n][:, :])
    V = {}
    V['avec'] = b.io.tile([128, 32], f32, name='avec', tag='avec')
    nc.sync.dma_start(out=V['avec'], in_=p['avec'][:, :])
    for n in ('conv_b', 'dt_b', 'dp_v'):
        V[n] = b.io.tile([128, 2], f32, name=n, tag=n)
        nc.sync.dma_start(out=V[n], in_=p[n][:, :])
    for n in ('bias_lm', 'bias_lg', 'bias_c', 'lo_b', 'ln_g', 'ln_b'):
        V[n] = b.io.tile([C, 1], f32, name=n, tag=n)
        nc.sync.dma_start(out=V[n], in_=p[n][:, :])
    ones_col = b.io.tile([C, 1], bf16, name='ones_col', tag='ones_col')
    nc.vector.memset(ones_col, 1.0)
    b.ones_col = ones_col
    ones_colf = b.io.tile([C, 1], f32, name='ones_colf', tag='ones_colf')
    nc.vector.memset(ones_colf, 1.0)
    ones_row = b.io.tile([1, 128], bf16, name='ones_row', tag='ones_row')
    nc.vector.memset(ones_row, 1.0)
    b.ones_row = ones_row
    eps_t = b.io.tile([1, 1], f32, name='lneps', tag='lneps')
    nc.vector.memset(eps_t, 1e-5)
    b.eps_t = eps_t

    # ---- P1: input layernorm ----
    nrm = b.pb.tile([C, L], bf16, name='nrm', tag='nrmo')
    _ln_stats_mm(b, x, ones_colf, nrm)

    # ---- P2 ----
    xmf = b.pb.tile([C, L], bf16, name='xmf', tag='xmf')
    _proj(b, W['wlmT'], nrm, xmf, AF.Identity, V['bias_lm'][:, :])
    xm_pad = b.pb.tile([C, D_CONV - 1 + L], bf16, name='xm_pad', tag='xm_pad')
    nc.vector.memset(xm_pad[:, 0:D_CONV - 1], 0.0)
    _proj(b, W['wcT'], xmf, xm_pad, AF.Silu, V['bias_c'][:, :],
          out_off=D_CONV - 1)

    uc = []
    for h in range(2):
        uct = b.pb.tile([128, L], bf16, name=f'uc{h}', tag=f'uc{h}')
        for pi in range(NPW):
            psu = b.ps.tile([128, PW], f32, name='bank', tag='bank')
            for half in range(2):
                base = pi * PW + half * CW
                for k in range(D_CONV):
                    nc.tensor.matmul(psu[:, half * CW:(half + 1) * CW],
                                     W[f'wk{h}{k}'],
                                     xm_pad[:, k + base:k + base + CW],
                                     start=(k == 0), stop=(k == D_CONV - 1))
            nc.scalar.activation(uct[:, pi * PW:(pi + 1) * PW], psu, AF.Silu,
                                 bias=V['conv_b'][:, h:h + 1])
        uc.append(uct)

    # dbl rows: 0-15 B, 16-31 C, 32-39 dtr (xp_w rows reordered host-side)
    dbl_sb = b.pb.tile([40, L], bf16, name='dbl_sb', tag='dbl_sb')
    dtr = dbl_sb[32:40, :]
    for pi in range(NPW):
        psd = b.ps.tile([40, PW], f32, name='bank', tag='bank')
        for half in range(2):
            hs = slice(half * CW, (half + 1) * CW)
            cs = slice(pi * PW + half * CW, pi * PW + (half + 1) * CW)
            nc.tensor.matmul(psd[:, hs], W['xpwT0'], uc[0][:, cs],
                             start=True, stop=False)
            nc.tensor.matmul(psd[:, hs], W['xpwT1'], uc[1][:, cs],
                             start=False, stop=True)
        nc.scalar.activation(dbl_sb[:, pi * PW:(pi + 1) * PW], psd,
                             AF.Identity, bias=0.0)
    bc_d = b.dram.tile([32, L], bf16, name='bc_d', tag='bc_d')
    nc.sync.dma_start(out=bc_d, in_=dbl_sb[0:32, :])

    # dt = ln(1 + exp(dt_w @ dtr + dt_b)) [f32]; dtu = dt * uc
    dt = []
    dtu = []
    for h in range(2):
        z1 = b.pf.tile([128, L], f32, name=f'z1{h}', tag='f')
        _proj(b, W[f'dtwT{h}'], dtr, z1, AF.Exp, V['dt_b'][:, h:h + 1],
              rows=128)
        dtt = b.pb.tile([128, L], f32, name=f'dt{h}', tag=f'dt{h}')
        nc.scalar.activation(dtt, z1, AF.Ln, bias=1.0, scale=1.0)
        dt.append(dtt)
        dtut = b.pb.tile([128, L], bf16, name=f'dtu{h}', tag=f'dtu{h}')
        nc.vector.scalar_tensor_tensor(dtut, dtt, 1.0, uc[h], mult, mult)
        dtu.append(dtut)

    sz = []
    yz = []

    def scan_block(h):
        psy = b.py.tile([128, L], f32, name='psy', tag='psy')
        for s in range(D_STATE):
            j = 16 * h + s
            b_bc = b.bc.tile([128, L], bf16, name='b_bc', tag='b_bc')
            src = bass.AP(tensor=bc_d.tensor, offset=bc_d.offset + s * L,
                          ap=[[0, 128], [1, L]])
            nc.sync.dma_start(out=b_bc, in_=src)
            c_bc = b.cb.tile([128, L], bf16, name='c_bc', tag='c_bc')
            src = bass.AP(tensor=bc_d.tensor, offset=bc_d.offset + (16 + s) * L,
                          ap=[[0, 128], [1, L]])
            nc.sync.dma_start(out=c_bc, in_=src)

            da = b.da.tile([128, L], f32, name='da', tag='da')
            nc.scalar.activation(da, dt[h], AF.Exp, bias=0.0,
                                 scale=V['avec'][:, j:j + 1])
            dbu = b.du.tile([128, L], bf16, name='dbu', tag='dbu')
            nc.vector.scalar_tensor_tensor(dbu, dtu[h], 1.0, b_bc, mult, mult)
            ht = b.ht.tile([128, L], bf16, name='ht', tag='ht')
            nc.vector.tensor_tensor_scan(ht, da, dbu, 0.0, mult, add)
            ycm = b.yc.tile([128, L], bf16, name='ycm', tag='ycm')
            nc.vector.scalar_tensor_tensor(ycm, ht, 1.0, c_bc, mult, mult)
            for ci in range(NCH):
                cs = slice(ci * CW, (ci + 1) * CW)
                nc.tensor.matmul(psy[:, cs], W['ident'], ycm[:, cs],
                                 start=(s == 0), stop=(s == D_STATE - 1),
                                 skip_group_check=True)
        return psy

    def yz_block(h, psy):
        yzt = b.pb.tile([128, L], bf16, name=f'yz{h}', tag=f'yz{h}')
        nc.vector.scalar_tensor_tensor(yzt, uc[h], V['dp_v'][:, h:h + 1],
                                       psy, mult, add)
        nc.vector.scalar_tensor_tensor(yzt, yzt, 1.0, sz[h], mult, mult)
        yz.append(yzt)

    def outproj_half(h, y_out):
        """y_out = owTA_h.T @ yz_h + owTB_h.T @ reversed(yz_h): per-core
        masked weights make this the direction-selected oriented output."""
        for pi in range(NPW):
            ps = b.ps.tile([C, PW], f32, name='bank', tag='bank')
            for half in range(2):
                hs = slice(half * CW, (half + 1) * CW)
                a0 = pi * PW + half * CW
                a1 = pi * PW + (half + 1) * CW
                nc.tensor.matmul(ps[:, hs], W[f'owTA{h}'], yz[h][:, a0:a1],
                                 start=True, stop=False)
                rcs = yz[h][:, L - a1:L - a0][:, ::-1]
                nc.tensor.matmul(ps[:, hs], W[f'owTB{h}'], rcs,
                                 start=False, stop=True)
            nc.scalar.activation(y_out[:, pi * PW:(pi + 1) * PW], ps,
                                 AF.Identity, bias=0.0)

    # h = 0 scans; gate/z projections run on ACT/PE meanwhile
    psy0 = scan_block(0)
    gate = b.pb.tile([C, L], bf16, name='gate', tag='gate')
    _proj(b, W['wlgT'], nrm, gate, AF.Silu, V['bias_lg'][:, :])
    for h in range(2):
        szt = b.pb.tile([128, L], bf16, name=f'sz{h}', tag=f'sz{h}')
        _proj(b, W[f'inwzT{h}'],
              xm_pad[:, D_CONV - 1:D_CONV - 1 + L], szt, AF.Silu, 0.0)
        sz.append(szt)
    yz_block(0, psy0)

    # h=0 out-projection + its AllReduce, hidden under the h=1 scan block
    y0 = b.pb.tile([C, L], bf16, name='y0', tag='y0')
    outproj_half(0, y0)
    cc_in0 = b.dram.tile([C, L], bf16, name='cc_in0', tag='cc_in0')
    cc_out0 = b.dram.tile([C, L], bf16, name='cc_out0', tag='cc_out0')
    nc.sync.dma_start(out=cc_in0, in_=y0)
    nc.gpsimd.collective_compute(
        'AllReduce', add,
        replica_groups=[[0, 1], [2, 3], [4, 5], [6, 7]],
        ins=[cc_in0.opt()], outs=[cc_out0.opt()])

    psy1 = scan_block(1)
    yz_block(1, psy1)

    y1t = b.pb.tile([C, L], bf16, name='y1t', tag='y1t')
    ysum2 = b.pb.tile([C, L], bf16, name='ysum2', tag='ysum2')
    for pi in range(NPW):
        hsl = slice(pi * PW, (pi + 1) * PW)
        for half in range(2):
            hs = slice(half * CW, (half + 1) * CW)
            a0 = pi * PW + half * CW
            a1 = pi * PW + (half + 1) * CW
            ps = b.ps.tile([C, PW], f32, name='bank', tag='bank') \
                if half == 0 else ps
            nc.tensor.matmul(ps[:, hs], W['owTA1'], yz[1][:, a0:a1],
                             start=True, stop=False)
            rcs = yz[1][:, L - a1:L - a0][:, ::-1]
            nc.tensor.matmul(ps[:, hs], W['owTB1'], rcs,
                             start=False, stop=True)
        nc.scalar.activation(y1t[:, hsl], ps, AF.Identity, bias=0.0)
        cc_in1 = b.dram.tile([C, PW], bf16, name=f'cc_in1{pi}', tag=f'cc_in1{pi}')
        cc_out1 = b.dram.tile([C, PW], bf16, name=f'cc_out1{pi}',
                              tag=f'cc_out1{pi}')
        nc.sync.dma_start(out=cc_in1, in_=y1t[:, hsl])
        nc.gpsimd.collective_compute(
            'AllReduce', add,
            replica_groups=[[0, 1], [2, 3], [4, 5], [6, 7]],
            ins=[cc_in1.opt()], outs=[cc_out1.opt()])
        nc.sync.dma_start(out=ysum2[:, hsl], in_=cc_out1)

    y_sum = b.pb.tile([C, L], bf16, name='y_sum', tag='y0')
    nc.sync.dma_start(out=y_sum, in_=cc_out0)

    # ---- P5 ----
    g1 = b.pb.tile([C, L], bf16, name='g1', tag='nrmo')
    nc.vector.scalar_tensor_tensor(g1, y_sum, 1.0, ysum2, mult, add)
    nc.vector.scalar_tensor_tensor(g1, g1, 1.0, gate, mult, mult)
    t2 = b.pb.tile([C, L], bf16, name='t2', tag='t2')
    _proj(b, W['loT'], g1, t2, AF.Identity, V['lo_b'][:, :])

    o1 = b.pb.tile([C, L], bf16, name='o1', tag='xm_pad')
    _ln_stats_mm(b, t2, b.ones_col, o1)
    out_sb = b.pf.tile([C, L], f32, name='out_sb', tag='f')
    nc.scalar.activation(out_sb, o1, AF.Identity, bias=V['ln_b'][:, :],
                         scale=V['ln_g'][:, :])
    nc.sync.dma_start(out=p['y'][:, :], in_=out_sb)


def _build_program():
    import contextlib
    nc = bacc.Bacc('TRN2', target_bir_lowering=False, debug=False, num_devices=8)
    p = _declare(nc)
    with tile.TileContext(nc) as tc:
        with contextlib.ExitStack() as ctx:
            _build_body(nc, tc, p, ctx)
    nc.compile()
    return nc


def _prep_core_inputs(inputs, bidx, d):
    g = lambda n: np.asarray(inputs[n], dtype=np.float32)
    x = g('x')
    ln_g = g('ln_g')
    ln_b = g('ln_b')
    pre = 'mf_' if d == 0 else 'mb_'
    P = lambda n: np.asarray(inputs[pre + n], dtype=np.float32)

    lm_w, lm_b = g('lm_w'), g('lm_b')
    lg_w, lg_b = g('lg_w'), g('lg_b')
    lo_w, lo_b = g('lo_w'), g('lo_b')
    if d == 0:
        wc, cb = g('cf_w'), g('cf_b')
    else:
        wc, cb = np.ascontiguousarray(g('cb_w')[:, ::-1]), g('cb_b')

    A = -np.exp(P('Alog'))
    avec = np.zeros((128, 32), np.float32)
    for h in range(2):
        for s in range(16):
            avec[:, 16 * h + s] = A[128 * h:128 * (h + 1), s]

    bf = lambda a: np.ascontiguousarray(np.asarray(a, dtype=ml_dtypes.bfloat16))
    col = lambda v: np.ascontiguousarray(v.astype(np.float32).reshape(-1, 1))
    halves = lambda v: np.ascontiguousarray(
        np.stack([v[:128], v[128:]], axis=1).astype(np.float32))
    T = lambda w: np.ascontiguousarray(w.T)

    in_w = P('in_w')
    conv_w = P('conv_w')
    xpw = P('xp_w')
    xpw = np.concatenate([xpw[DT_RANK:], xpw[:DT_RANK]], axis=0)
    xpwT = np.ascontiguousarray(xpw.T)
    outwT = np.ascontiguousarray(P('out_w').T)
    dtwT = np.ascontiguousarray(P('dt_w').T)

    out = {
        'x': np.ascontiguousarray(x[bidx]),
        'wlmT': bf(T(lm_w * ln_g[None, :])),
        'wlgT': bf(T(lg_w * ln_g[None, :])),
        'wcT': bf(T(wc)),
        'loT': bf(T(lo_w)),
        'ident': bf(np.eye(128, dtype=np.float32)),
        'avec': avec,
        'conv_b': halves(P('conv_b')),
        'dt_b': halves(P('dt_b')),
        'dp_v': halves(P('D')),
        'bias_lm': col(lm_w @ ln_b + lm_b),
        'bias_lg': col(lg_w @ ln_b + lg_b),
        'bias_c': col(cb),
        'lo_b': col(lo_b),
        'ln_g': col(ln_g),
        'ln_b': col(ln_b),
    }
    for h in range(2):
        hsl = slice(128 * h, 128 * (h + 1))
        for k in range(D_CONV):
            wk = in_w[hsl, :] * conv_w[hsl, k:k + 1]
            out[f'wk{h}{k}'] = bf(T(wk))
        out[f'inwzT{h}'] = bf(T(P('in_w')[256:][hsl, :]))
        ow = outwT[hsl, :]
        out[f'owTA{h}'] = bf(ow if d == 0 else np.zeros_like(ow))
        out[f'owTB{h}'] = bf(np.zeros_like(ow) if d == 0 else ow)
        out[f'xpwT{h}'] = bf(xpwT[hsl, :])
        out[f'dtwT{h}'] = bf(dtwT[:, hsl])
    return out


def get_program():
    global _PROGRAM
    if _PROGRAM is None:
        _PROGRAM = _build_program()
    return _PROGRAM


def run(inputs, **run_kwargs):
    nc = get_program()
    in_maps = [_prep_core_inputs(inputs, c // 2, c % 2) for c in range(8)]
    res = run_bass_kernel_spmd(nc, in_maps, core_ids=list(range(8)), **run_kwargs)
    out = np.stack([res.results[2 * b]['y'] for b in range(BATCH)], axis=0)
    return out, res


def kernel(**inputs) -> np.ndarray:
    out, _ = run(inputs)
    return out.astype(np.float32)



# revision 11
# speedup vs baseline: 1.1939x; 1.1939x over previous
"""Bidirectional Mamba block (BiT_MamSleep) on 8 TRN2 NeuronCores — v6.

Sharding: core c handles (batch b = c//2, direction dir = c%2); pairwise
AllReduce joins the two directions; both cores compute the tail redundantly.

s-major scan layout: 32 tiles of [128 part = d (one half of d_inner),
free = t], one per (half h, state s).  dA_s comes straight from ACT exp with
per-partition scale A[:, s]; B/C are row-broadcast per state; the sum over s
is identity-matmul PSUM accumulation on PE.  The depthwise conv is folded
into the in-projection (4 shifted-AP matmuls).  All matmuls bf16.

All elementwise work runs on DVE: GpSimd tensor ops contend with DVE for the
shared SBUF port and slow concurrent DVE instructions by ~80% (measured), so
GpSimd only drives the collectives.  The direction flip + select is folded
into the out-projection via per-core masked forward/reversed weights.  The
exchange is split per half h: the h=0 AllReduce runs hidden under the h=1
scan block.
"""
import sys

if '/opt/trn_rl_repo' not in sys.path:
    sys.path.insert(0, '/opt/trn_rl_repo')

import ml_dtypes
import numpy as np

import concourse.bass as bass
import concourse.bacc as bacc
import concourse.tile as tile
from concourse import mybir
from concourse.bass_utils import run_bass_kernel_spmd

HID = 128
BATCH = 4
SEQ = 2048
D_STATE = 16
D_CONV = 4
D_INNER = 256
DT_RANK = 8

L = SEQ
C = HID
CW = 512
NCH = L // CW
PW = 1024
NPW = L // PW
f32 = mybir.dt.float32
bf16 = mybir.dt.bfloat16
mult = mybir.AluOpType.mult
add = mybir.AluOpType.add
sub = mybir.AluOpType.subtract
AF = mybir.ActivationFunctionType

_PROGRAM = None


def _declare(nc):
    dpf = lambda name, shape: nc.declare_dram_parameter(name, list(shape), f32,
                                                        isOutput=False)
    dph = lambda name, shape: nc.declare_dram_parameter(name, list(shape), bf16,
                                                        isOutput=False)
    p = {}
    p['x'] = dpf('x', (C, L))
    for n in ('wlmT', 'wlgT', 'wcT', 'loT'):
        p[n] = dph(n, (C, C))
    for h in range(2):
        for k in range(D_CONV):
            p[f'wk{h}{k}'] = dph(f'wk{h}{k}', (C, C))
        p[f'inwzT{h}'] = dph(f'inwzT{h}', (C, C))
        p[f'owTA{h}'] = dph(f'owTA{h}', (128, C))
        p[f'owTB{h}'] = dph(f'owTB{h}', (128, C))
        p[f'xpwT{h}'] = dph(f'xpwT{h}', (128, DT_RANK + 2 * D_STATE))
        p[f'dtwT{h}'] = dph(f'dtwT{h}', (DT_RANK, 128))
    p['ident'] = dph('ident', (128, 128))
    for h in range(2):
        p[f'diagD{h}'] = dph(f'diagD{h}', (128, 128))
    p['avec'] = dpf('avec', (128, 32))
    for n in ('conv_b', 'dt_b'):
        p[n] = dpf(n, (128, 2))
    for n in ('bias_lm', 'bias_lg', 'bias_c', 'lo_b', 'ln_g', 'ln_b'):
        p[n] = dpf(n, (C, 1))
    p['y'] = nc.declare_dram_parameter('y', [C, L], f32, isOutput=True)
    return p


class B:
    pass


def _ln_stats_mm(b, x_sb, ones_in, out_bf):
    """LayerNorm over the 128 channels per column; stage-major to keep the
    ACT table set stable."""
    nc = b.nc
    rows = b.pf.tile([128, L], f32, name='lnrows', tag='f')
    rows_bf = b.pb.tile([2, L], bf16, name='lnrowsb', tag='lnb')
    ex = rows_bf[0:1, :]
    nrm0 = b.pb.tile([C, L], bf16, name='nrm0', tag='xmf')
    sq2 = b.pb.tile([C, L], bf16, name='sq2', tag='y0')
    rr = rows[0:1, :]
    csl = [slice(ci * CW, (ci + 1) * CW) for ci in range(NCH)]
    for cs in csl:
        ps0 = b.ps.tile([1, CW], f32, name='bank', tag='bank')
        nc.tensor.matmul(ps0, ones_in, x_sb[:, cs], start=True, stop=True)
        nc.scalar.activation(ex[:, cs], ps0, AF.Identity, bias=0.0, scale=1.0 / C)
    for cs in csl:
        psb = b.ps.tile([128, CW], f32, name='bank', tag='bank')
        nc.tensor.matmul(psb, b.ones_row, ex[:, cs], start=True, stop=True)
        nc.vector.scalar_tensor_tensor(nrm0[:, cs], x_sb[:, cs], 1.0, psb,
                                       mult, sub)
    for cs in csl:
        nc.scalar.activation(sq2[:, cs], nrm0[:, cs], AF.Square)
    for cs in csl:
        psv = b.ps.tile([1, CW], f32, name='bank', tag='bank')
        nc.tensor.matmul(psv, b.ones_col, sq2[:, cs], start=True, stop=True)
        nc.scalar.activation(rr[:, cs], psv, AF.Ln, bias=b.eps_t[:, :],
                             scale=1.0 / C)
    for cs in csl:
        nc.scalar.activation(ex[:, cs], rr[:, cs], AF.Exp, bias=0.0, scale=-0.5)
    for cs in csl:
        psr = b.ps.tile([128, CW], f32, name='bank', tag='bank')
        nc.tensor.matmul(psr, b.ones_row, ex[:, cs], start=True, stop=True)
        nc.vector.scalar_tensor_tensor(out_bf[:, cs], nrm0[:, cs], 1.0, psr,
                                       mult, mult)


def _proj(b, lhsT, rhs, out, func, bias, rows=C, out_off=0):
    nc = b.nc
    for pi in range(NPW):
        ps = b.ps.tile([rows, PW], f32, name='bank', tag='bank')
        for half in range(2):
            cs = slice(pi * PW + half * CW, pi * PW + (half + 1) * CW)
            nc.tensor.matmul(ps[:, half * CW:(half + 1) * CW], lhsT, rhs[:, cs],
                             start=True, stop=True)
        ocs = slice(out_off + pi * PW, out_off + (pi + 1) * PW)
        nc.scalar.activation(out[:, ocs], ps, func, bias=bias)


def _build_body(nc, tc, p, ctx):
    b = B()
    b.nc = nc
    b.io = ctx.enter_context(tc.tile_pool(name='io', bufs=1))
    b.pb = ctx.enter_context(tc.tile_pool(name='pb', bufs=1))
    b.pf = ctx.enter_context(tc.tile_pool(name='pf', bufs=3))
    b.bc = ctx.enter_context(tc.tile_pool(name='bc', bufs=3))
    b.cb = ctx.enter_context(tc.tile_pool(name='cb', bufs=3))
    b.da = ctx.enter_context(tc.tile_pool(name='da', bufs=3))
    b.du = ctx.enter_context(tc.tile_pool(name='du', bufs=3))
    b.ht = ctx.enter_context(tc.tile_pool(name='ht', bufs=3))
    b.yc = ctx.enter_context(tc.tile_pool(name='yc', bufs=3))
    b.ps = ctx.enter_context(tc.tile_pool(name='ps', bufs=2, space='PSUM'))
    b.py = ctx.enter_context(tc.tile_pool(name='py', bufs=1, space='PSUM'))
    b.dram = ctx.enter_context(tc.tile_pool(name='drm', bufs=1, space='DRAM'))

    x = b.pf.tile([C, L], f32, name='x', tag='f')
    nc.sync.dma_start(out=x, in_=p['x'][:, :])

    W = {}
    wspec = [('wlmT', (C, C)), ('wlgT', (C, C)), ('wcT', (C, C)),
             ('loT', (C, C)), ('ident', (128, 128)),
             ('diagD0', (128, 128)), ('diagD1', (128, 128))]
    for h in range(2):
        wspec += [(f'wk{h}{k}', (C, C)) for k in range(D_CONV)]
        wspec += [(f'inwzT{h}', (C, C)), (f'owTA{h}', (128, C)),
                  (f'owTB{h}', (128, C)),
                  (f'xpwT{h}', (128, 40)), (f'dtwT{h}', (8, 128))]
    for n, shape in wspec:
        if n.startswith('dtwT'):
            W[n] = b.io.tile([40, shape[1]], bf16, name=n, tag=n)
            nc.sync.dma_start(out=W[n][32:40, :], in_=p[n][:, :])
            W[n] = W[n][32:40, :]
        else:
            W[n] = b.io.tile(list(shape), bf16, name=n, tag=n)
            nc.sync.dma_start(out=W[n], in_=p[n][:, :])
    V = {}
    V['avec'] = b.io.tile([128, 32], f32, name='avec', tag='avec')
    nc.sync.dma_start(out=V['avec'], in_=p['avec'][:, :])
    for n in ('conv_b', 'dt_b'):
        V[n] = b.io.tile([128, 2], f32, name=n, tag=n)
        nc.sync.dma_start(out=V[n], in_=p[n][:, :])
    for n in ('bias_lm', 'bias_lg', 'bias_c', 'lo_b', 'ln_g', 'ln_b'):
        V[n] = b.io.tile([C, 1], f32, name=n, tag=n)
        nc.sync.dma_start(out=V[n], in_=p[n][:, :])
    ones_col = b.io.tile([C, 1], bf16, name='ones_col', tag='ones_col')
    nc.vector.memset(ones_col, 1.0)
    b.ones_col = ones_col
    ones_colf = b.io.tile([C, 1], f32, name='ones_colf', tag='ones_colf')
    nc.vector.memset(ones_colf, 1.0)
    ones_row = b.io.tile([1, 128], bf16, name='ones_row', tag='ones_row')
    nc.vector.memset(ones_row, 1.0)
    b.ones_row = ones_row
    eps_t = b.io.tile([1, 1], f32, name='lneps', tag='lneps')
    nc.vector.memset(eps_t, 1e-5)
    b.eps_t = eps_t

    # ---- P1: input layernorm ----
    nrm = b.pb.tile([C, L], bf16, name='nrm', tag='nrmo')
    _ln_stats_mm(b, x, ones_colf, nrm)

    # ---- P2 ----
    xmf = b.pb.tile([C, L], bf16, name='xmf', tag='xmf')
    _proj(b, W['wlmT'], nrm, xmf, AF.Identity, V['bias_lm'][:, :])
    xm_pad = b.pb.tile([C, D_CONV - 1 + L], bf16, name='xm_pad', tag='xm_pad')
    nc.vector.memset(xm_pad[:, 0:D_CONV - 1], 0.0)
    _proj(b, W['wcT'], xmf, xm_pad, AF.Silu, V['bias_c'][:, :],
          out_off=D_CONV - 1)

    uc = []
    for h in range(2):
        uct = b.pb.tile([128, L], bf16, name=f'uc{h}', tag=f'uc{h}')
        for pi in range(NPW):
            psu = b.ps.tile([128, PW], f32, name='bank', tag='bank')
            for half in range(2):
                base = pi * PW + half * CW
                for k in range(D_CONV):
                    nc.tensor.matmul(psu[:, half * CW:(half + 1) * CW],
                                     W[f'wk{h}{k}'],
                                     xm_pad[:, k + base:k + base + CW],
                                     start=(k == 0), stop=(k == D_CONV - 1))
            nc.scalar.activation(uct[:, pi * PW:(pi + 1) * PW], psu, AF.Silu,
                                 bias=V['conv_b'][:, h:h + 1])
        uc.append(uct)

    # dbl rows: 0-15 B, 16-31 C, 32-39 dtr (xp_w rows reordered host-side)
    dbl_sb = b.pb.tile([40, L], bf16, name='dbl_sb', tag='dbl_sb')
    dtr = dbl_sb[32:40, :]
    for pi in range(NPW):
        psd = b.ps.tile([40, PW], f32, name='bank', tag='bank')
        for half in range(2):
            hs = slice(half * CW, (half + 1) * CW)
            cs = slice(pi * PW + half * CW, pi * PW + (half + 1) * CW)
            nc.tensor.matmul(psd[:, hs], W['xpwT0'], uc[0][:, cs],
                             start=True, stop=False)
            nc.tensor.matmul(psd[:, hs], W['xpwT1'], uc[1][:, cs],
                             start=False, stop=True)
        nc.scalar.activation(dbl_sb[:, pi * PW:(pi + 1) * PW], psd,
                             AF.Identity, bias=0.0)
    bc_d = b.dram.tile([32, L], bf16, name='bc_d', tag='bc_d')
    nc.sync.dma_start(out=bc_d, in_=dbl_sb[0:32, :])

    # dt = ln(1 + exp(dt_w @ dtr + dt_b)) [f32]; dtu = dt * uc
    dt = []
    dtu = []
    for h in range(2):
        z1 = b.pf.tile([128, L], f32, name=f'z1{h}', tag='f')
        _proj(b, W[f'dtwT{h}'], dtr, z1, AF.Exp, V['dt_b'][:, h:h + 1],
              rows=128)
        dtt = b.pb.tile([128, L], f32, name=f'dt{h}', tag=f'dt{h}')
        nc.scalar.activation(dtt, z1, AF.Ln, bias=1.0, scale=1.0)
        dt.append(dtt)
        dtut = b.pb.tile([128, L], bf16, name=f'dtu{h}', tag=f'dtu{h}')
        nc.vector.tensor_tensor(dtut, dtt, uc[h], mult)
        dtu.append(dtut)

    sz = []
    yz = []

    def scan_block(h):
        psy = b.py.tile([128, L], f32, name='psy', tag='psy')
        for s in range(D_STATE):
            j = 16 * h + s
            b_bc = b.bc.tile([128, L], bf16, name='b_bc', tag='b_bc')
            src = bass.AP(tensor=bc_d.tensor, offset=bc_d.offset + s * L,
                          ap=[[0, 128], [1, L]])
            nc.sync.dma_start(out=b_bc, in_=src)
            c_bc = b.cb.tile([128, L], bf16, name='c_bc', tag='c_bc')
            src = bass.AP(tensor=bc_d.tensor, offset=bc_d.offset + (16 + s) * L,
                          ap=[[0, 128], [1, L]])
            nc.sync.dma_start(out=c_bc, in_=src)

            da = b.da.tile([128, L], f32, name='da', tag='da')
            nc.scalar.activation(da, dt[h], AF.Exp, bias=0.0,
                                 scale=V['avec'][:, j:j + 1])
            dbu = b.du.tile([128, L], bf16, name='dbu', tag='dbu')
            nc.vector.tensor_tensor(dbu, dtu[h], b_bc, mult)
            ht = b.ht.tile([128, L], bf16, name='ht', tag='ht')
            nc.vector.tensor_tensor_scan(ht, da, dbu, 0.0, mult, add)
            ycm = b.yc.tile([128, L], bf16, name='ycm', tag='ycm')
            nc.vector.tensor_tensor(ycm, ht, c_bc, mult)
            for ci in range(NCH):
                cs = slice(ci * CW, (ci + 1) * CW)
                nc.tensor.matmul(psy[:, cs], W['ident'], ycm[:, cs],
                                 start=(s == 0), stop=False,
                                 skip_group_check=True)
        # fold uc * D into psy on PE (diagonal weights), closing the group
        for ci in range(NCH):
            cs = slice(ci * CW, (ci + 1) * CW)
            nc.tensor.matmul(psy[:, cs], W[f'diagD{h}'], uc[h][:, cs],
                             start=False, stop=True, skip_group_check=True)
        return psy

    def yz_block(h, psy):
        # reuses dtu[h]'s buffer: dtu[h]'s last read is state 15's dbu
        yq = b.pb.tile([128, L], bf16, name=f'yq{h}', tag=f'dtu{h}')
        nc.scalar.activation(yq, psy, AF.Identity, bias=0.0)
        yzt = b.pb.tile([128, L], bf16, name=f'yz{h}', tag=f'yz{h}')
        nc.vector.tensor_tensor(yzt, yq, sz[h], mult)
        yz.append(yzt)

    def outproj_half(h, y_out):
        """y_out = owTA_h.T @ yz_h + owTB_h.T @ reversed(yz_h): per-core
        masked weights make this the direction-selected oriented output."""
        for pi in range(NPW):
            ps = b.ps.tile([C, PW], f32, name='bank', tag='bank')
            for half in range(2):
                hs = slice(half * CW, (half + 1) * CW)
                a0 = pi * PW + half * CW
                a1 = pi * PW + (half + 1) * CW
                nc.tensor.matmul(ps[:, hs], W[f'owTA{h}'], yz[h][:, a0:a1],
                                 start=True, stop=False)
                rcs = yz[h][:, L - a1:L - a0][:, ::-1]
                nc.tensor.matmul(ps[:, hs], W[f'owTB{h}'], rcs,
                                 start=False, stop=True)
            nc.scalar.activation(y_out[:, pi * PW:(pi + 1) * PW], ps,
                                 AF.Identity, bias=0.0)

    # h = 0 scans; gate/z projections run on ACT/PE meanwhile
    psy0 = scan_block(0)
    gate = b.pb.tile([C, L], bf16, name='gate', tag='gate')
    _proj(b, W['wlgT'], nrm, gate, AF.Silu, V['bias_lg'][:, :])
    for h in range(2):
        szt = b.pb.tile([128, L], bf16, name=f'sz{h}', tag=f'sz{h}')
        _proj(b, W[f'inwzT{h}'],
              xm_pad[:, D_CONV - 1:D_CONV - 1 + L], szt, AF.Silu, 0.0)
        sz.append(szt)
    yz_block(0, psy0)

    # h=0 out-projection + its AllReduce, hidden under the h=1 scan block
    y0 = b.pb.tile([C, L], bf16, name='y0', tag='y0')
    outproj_half(0, y0)
    cc_in0 = b.dram.tile([C, L], bf16, name='cc_in0', tag='cc_in0')
    cc_out0 = b.dram.tile([C, L], bf16, name='cc_out0', tag='cc_out0')
    nc.sync.dma_start(out=cc_in0, in_=y0)
    nc.gpsimd.collective_compute(
        'AllReduce', add,
        replica_groups=[[0, 1], [2, 3], [4, 5], [6, 7]],
        ins=[cc_in0.opt()], outs=[cc_out0.opt()])

    psy1 = scan_block(1)
    yz_block(1, psy1)

    y1t = b.pb.tile([C, L], bf16, name='y1t', tag='y1t')
    outproj_half(1, y1t)
    ysum2 = b.pb.tile([C, L], bf16, name='ysum2', tag='ysum2')
    cc_in1 = b.dram.tile([C, L], bf16, name='cc_in1', tag='cc_in1')
    cc_out1 = b.dram.tile([C, L], bf16, name='cc_out1', tag='cc_out1')
    nc.sync.dma_start(out=cc_in1, in_=y1t)
    nc.gpsimd.collective_compute(
        'AllReduce', add,
        replica_groups=[[0, 1], [2, 3], [4, 5], [6, 7]],
        ins=[cc_in1.opt()], outs=[cc_out1.opt()])
    nc.sync.dma_start(out=ysum2, in_=cc_out1)

    y_sum = b.pb.tile([C, L], bf16, name='y_sum', tag='y0')
    nc.sync.dma_start(out=y_sum, in_=cc_out0)

    # ---- P5 ----
    g1 = b.pb.tile([C, L], bf16, name='g1', tag='nrmo')
    nc.vector.tensor_tensor(g1, y_sum, ysum2, add)
    nc.vector.tensor_tensor(g1, g1, gate, mult)
    t2 = b.pb.tile([C, L], bf16, name='t2', tag='t2')
    _proj(b, W['loT'], g1, t2, AF.Identity, V['lo_b'][:, :])

    o1 = b.pb.tile([C, L], bf16, name='o1', tag='xm_pad')
    _ln_stats_mm(b, t2, b.ones_col, o1)
    out_sb = b.pf.tile([C, L], f32, name='out_sb', tag='f')
    nc.scalar.activation(out_sb, o1, AF.Identity, bias=V['ln_b'][:, :],
                         scale=V['ln_g'][:, :])
    nc.sync.dma_start(out=p['y'][:, :], in_=out_sb)


def _build_program():
    import contextlib
    nc = bacc.Bacc('TRN2', target_bir_lowering=False, debug=False, num_devices=8)
    p = _declare(nc)
    with tile.TileContext(nc) as tc:
        with contextlib.ExitStack() as ctx:
            _build_body(nc, tc, p, ctx)
    nc.compile()
    return nc


def _prep_core_inputs(inputs, bidx, d):
    g = lambda n: np.asarray(inputs[n], dtype=np.float32)
    x = g('x')
    ln_g = g('ln_g')
    ln_b = g('ln_b')
    pre = 'mf_' if d == 0 else 'mb_'
    P = lambda n: np.asarray(inputs[pre + n], dtype=np.float32)

    lm_w, lm_b = g('lm_w'), g('lm_b')
    lg_w, lg_b = g('lg_w'), g('lg_b')
    lo_w, lo_b = g('lo_w'), g('lo_b')
    if d == 0:
        wc, cb = g('cf_w'), g('cf_b')
    else:
        wc, cb = np.ascontiguousarray(g('cb_w')[:, ::-1]), g('cb_b')

    A = -np.exp(P('Alog'))
    avec = np.zeros((128, 32), np.float32)
    for h in range(2):
        for s in range(16):
            avec[:, 16 * h + s] = A[128 * h:128 * (h + 1), s]

    bf = lambda a: np.ascontiguousarray(np.asarray(a, dtype=ml_dtypes.bfloat16))
    col = lambda v: np.ascontiguousarray(v.astype(np.float32).reshape(-1, 1))
    halves = lambda v: np.ascontiguousarray(
        np.stack([v[:128], v[128:]], axis=1).astype(np.float32))
    T = lambda w: np.ascontiguousarray(w.T)

    in_w = P('in_w')
    conv_w = P('conv_w')
    xpw = P('xp_w')
    xpw = np.concatenate([xpw[DT_RANK:], xpw[:DT_RANK]], axis=0)
    xpwT = np.ascontiguousarray(xpw.T)
    outwT = np.ascontiguousarray(P('out_w').T)
    dtwT = np.ascontiguousarray(P('dt_w').T)

    out = {
        'x': np.ascontiguousarray(x[bidx]),
        'wlmT': bf(T(lm_w * ln_g[None, :])),
        'wlgT': bf(T(lg_w * ln_g[None, :])),
        'wcT': bf(T(wc)),
        'loT': bf(T(lo_w)),
        'ident': bf(np.eye(128, dtype=np.float32)),
        'avec': avec,
        'conv_b': halves(P('conv_b')),
        'dt_b': halves(P('dt_b')),
        'bias_lm': col(lm_w @ ln_b + lm_b),
        'bias_lg': col(lg_w @ ln_b + lg_b),
        'bias_c': col(cb),
        'lo_b': col(lo_b),
        'ln_g': col(ln_g),
        'ln_b': col(ln_b),
    }
    for h in range(2):
        hsl = slice(128 * h, 128 * (h + 1))
        out[f'diagD{h}'] = bf(np.diag(P('D')[hsl]).astype(np.float32))
        for k in range(D_CONV):
            wk = in_w[hsl, :] * conv_w[hsl, k:k + 1]
            out[f'wk{h}{k}'] = bf(T(wk))
        out[f'inwzT{h}'] = bf(T(P('in_w')[256:][hsl, :]))
        ow = outwT[hsl, :]
        out[f'owTA{h}'] = bf(ow if d == 0 else np.zeros_like(ow))
        out[f'owTB{h}'] = bf(np.zeros_like(ow) if d == 0 else ow)
        out[f'xpwT{h}'] = bf(xpwT[hsl, :])
        out[f'dtwT{h}'] = bf(dtwT[:, hsl])
    return out


def get_program():
    global _PROGRAM
    if _PROGRAM is None:
        _PROGRAM = _build_program()
    return _PROGRAM


def run(inputs, **run_kwargs):
    nc = get_program()
    in_maps = [_prep_core_inputs(inputs, c // 2, c % 2) for c in range(8)]
    res = run_bass_kernel_spmd(nc, in_maps, core_ids=list(range(8)), **run_kwargs)
    out = np.stack([res.results[2 * b]['y'] for b in range(BATCH)], axis=0)
    return out, res


def kernel(**inputs) -> np.ndarray:
    out, _ = run(inputs)
    return out.astype(np.float32)



# revision 21
# speedup vs baseline: 1.2714x; 1.0649x over previous
"""Bidirectional Mamba block (BiT_MamSleep) on 8 TRN2 NeuronCores — v6.

Sharding: core c handles (batch b = c//2, direction dir = c%2); pairwise
AllReduce joins the two directions; both cores compute the tail redundantly.

s-major scan layout: 32 tiles of [128 part = d (one half of d_inner),
free = t], one per (half h, state s).  dA_s comes straight from ACT exp with
per-partition scale A[:, s]; B/C are row-broadcast per state; the sum over s
is identity-matmul PSUM accumulation on PE.  The depthwise conv is folded
into the in-projection (4 shifted-AP matmuls).  All matmuls bf16.

All elementwise work runs on DVE: GpSimd tensor ops contend with DVE for the
shared SBUF port and slow concurrent DVE instructions by ~80% (measured), so
GpSimd only drives the collectives.  The direction flip + select is folded
into the out-projection via per-core masked forward/reversed weights.  The
exchange is split per half h: the h=0 AllReduce runs hidden under the h=1
scan block.
"""
import sys

if '/opt/trn_rl_repo' not in sys.path:
    sys.path.insert(0, '/opt/trn_rl_repo')

import ml_dtypes
import numpy as np

import concourse.bass as bass
import concourse.bacc as bacc
import concourse.tile as tile
from concourse import mybir
from concourse.bass_utils import run_bass_kernel_spmd

HID = 128
BATCH = 4
SEQ = 2048
D_STATE = 16
D_CONV = 4
D_INNER = 256
DT_RANK = 8

L = SEQ
C = HID
CW = 512
NCH = L // CW
PW = 1024
NPW = L // PW
f32 = mybir.dt.float32
bf16 = mybir.dt.bfloat16
mult = mybir.AluOpType.mult
add = mybir.AluOpType.add
sub = mybir.AluOpType.subtract
AF = mybir.ActivationFunctionType

_PROGRAM = None


def _declare(nc):
    dpf = lambda name, shape: nc.declare_dram_parameter(name, list(shape), f32,
                                                        isOutput=False)
    dph = lambda name, shape: nc.declare_dram_parameter(name, list(shape), bf16,
                                                        isOutput=False)
    p = {}
    p['x'] = dpf('x', (C, L))
    for n in ('wlmT', 'wlgT', 'wcT', 'loT'):
        p[n] = dph(n, (C, C))
    for h in range(2):
        for k in range(D_CONV):
            p[f'wk{h}{k}'] = dph(f'wk{h}{k}', (C, C))
        p[f'inwzT{h}'] = dph(f'inwzT{h}', (C, C))
        p[f'owTA{h}'] = dph(f'owTA{h}', (128, C))
        p[f'owTB{h}'] = dph(f'owTB{h}', (128, C))
        p[f'xpwT{h}'] = dph(f'xpwT{h}', (128, DT_RANK + 2 * D_STATE))
        p[f'dtwT{h}'] = dph(f'dtwT{h}', (DT_RANK, 128))
    p['ident'] = dph('ident', (128, 128))
    for h in range(2):
        p[f'diagD{h}'] = dph(f'diagD{h}', (128, 128))
    p['avec'] = dpf('avec', (128, 32))
    for n in ('conv_b', 'dt_b'):
        p[n] = dpf(n, (128, 2))
    for n in ('bias_lm', 'bias_lg', 'bias_c', 'lo_b', 'ln_g', 'ln_b'):
        p[n] = dpf(n, (C, 1))
    p['y'] = nc.declare_dram_parameter('y', [C, L], f32, isOutput=True)
    return p


class B:
    pass


def _ln_stats_mm(b, x_sb, ones_in, out_bf):
    """LayerNorm over the 128 channels per column; stage-major to keep the
    ACT table set stable."""
    nc = b.nc
    rows_bf = b.pb.tile([2, L], bf16, name='lnrowsb', tag='lnb')
    rows_f = b.pb.tile([1, L], f32, name='lnrowsf', tag='lnf')
    ex = rows_bf[0:1, :]
    nrm0 = b.pb.tile([C, L], bf16, name='nrm0', tag='xmf')
    sq2 = b.pb.tile([C, L], bf16, name='sq2', tag='y0')
    csl = [slice(ci * CW, (ci + 1) * CW) for ci in range(NCH)]
    for cs in csl:
        ps0 = b.ps.tile([1, CW], f32, name='bank', tag='bank')
        nc.tensor.matmul(ps0, ones_in, x_sb[:, cs], start=True, stop=True)
        nc.scalar.activation(ex[:, cs], ps0, AF.Identity, bias=0.0, scale=1.0 / C)
    for cs in csl:
        psb = b.ps.tile([128, CW], f32, name='bank', tag='bank')
        nc.tensor.matmul(psb, b.ones_row, ex[:, cs], start=True, stop=True)
        nc.vector.scalar_tensor_tensor(nrm0[:, cs], x_sb[:, cs], 1.0, psb,
                                       mult, sub)
    for cs in csl:
        nc.scalar.activation(sq2[:, cs], nrm0[:, cs], AF.Square)
    rr = rows_f[0:1, :]
    for cs in csl:
        psv = b.ps.tile([1, CW], f32, name='bank', tag='bank')
        nc.tensor.matmul(psv, b.ones_col, sq2[:, cs], start=True, stop=True)
        nc.scalar.activation(rr[:, cs], psv, AF.Ln, bias=b.eps_t[:, :],
                             scale=1.0 / C)
    for cs in csl:
        nc.scalar.activation(ex[:, cs], rr[:, cs], AF.Exp, bias=0.0, scale=-0.5)
    for cs in csl:
        psr = b.ps.tile([128, CW], f32, name='bank', tag='bank')
        nc.tensor.matmul(psr, b.ones_row, ex[:, cs], start=True, stop=True)
        nc.vector.scalar_tensor_tensor(out_bf[:, cs], nrm0[:, cs], 1.0, psr,
                                       mult, mult)


def _proj(b, lhsT, rhs, out, func, bias, rows=C, out_off=0):
    nc = b.nc
    for pi in range(NPW):
        ps = b.ps.tile([rows, PW], f32, name='bank', tag='bank')
        for half in range(2):
            cs = slice(pi * PW + half * CW, pi * PW + (half + 1) * CW)
            nc.tensor.matmul(ps[:, half * CW:(half + 1) * CW], lhsT, rhs[:, cs],
                             start=True, stop=True)
        ocs = slice(out_off + pi * PW, out_off + (pi + 1) * PW)
        nc.scalar.activation(out[:, ocs], ps, func, bias=bias)


def _build_body(nc, tc, p, ctx):
    b = B()
    b.nc = nc
    b.io = ctx.enter_context(tc.tile_pool(name='io', bufs=1))
    b.pb = ctx.enter_context(tc.tile_pool(name='pb', bufs=1))
    b.pf = ctx.enter_context(tc.tile_pool(name='pf', bufs=2))
    b.bc = ctx.enter_context(tc.tile_pool(name='bc', bufs=3))
    b.cb = ctx.enter_context(tc.tile_pool(name='cb', bufs=3))
    b.da = ctx.enter_context(tc.tile_pool(name='da', bufs=2))
    b.du = ctx.enter_context(tc.tile_pool(name='du', bufs=3))
    b.ht = ctx.enter_context(tc.tile_pool(name='ht', bufs=3))
    b.yc = ctx.enter_context(tc.tile_pool(name='yc', bufs=3))
    b.ps = ctx.enter_context(tc.tile_pool(name='ps', bufs=2, space='PSUM'))
    b.py = ctx.enter_context(tc.tile_pool(name='py', bufs=1, space='PSUM'))
    b.dram = ctx.enter_context(tc.tile_pool(name='drm', bufs=1, space='DRAM'))

    x = b.pf.tile([C, L], f32, name='x', tag='f')
    for ci in range(NCH):
        cs = slice(ci * CW, (ci + 1) * CW)
        nc.sync.dma_start(out=x[:, cs], in_=p['x'][:, cs])

    W = {}
    wspec = [('wlmT', (C, C)), ('wlgT', (C, C)), ('wcT', (C, C)),
             ('loT', (C, C)), ('ident', (128, 128)),
             ('diagD0', (128, 128)), ('diagD1', (128, 128))]
    for h in range(2):
        wspec += [(f'wk{h}{k}', (C, C)) for k in range(D_CONV)]
        wspec += [(f'inwzT{h}', (C, C)), (f'owTA{h}', (128, C)),
                  (f'owTB{h}', (128, C)),
                  (f'xpwT{h}', (128, 40)), (f'dtwT{h}', (8, 128))]
    for n, shape in wspec:
        if n.startswith('dtwT'):
            W[n] = b.io.tile([40, shape[1]], bf16, name=n, tag=n)
            nc.sync.dma_start(out=W[n][32:40, :], in_=p[n][:, :])
            W[n] = W[n][32:40, :]
        else:
            W[n] = b.io.tile(list(shape), bf16, name=n, tag=n)
            nc.sync.dma_start(out=W[n], in_=p[n][:, :])
    V = {}
    V['avec'] = b.io.tile([128, 32], f32, name='avec', tag='avec')
    nc.sync.dma_start(out=V['avec'], in_=p['avec'][:, :])
    for n in ('conv_b', 'dt_b'):
        V[n] = b.io.tile([128, 2], f32, name=n, tag=n)
        nc.sync.dma_start(out=V[n], in_=p[n][:, :])
    for n in ('bias_lm', 'bias_lg', 'bias_c', 'lo_b', 'ln_g', 'ln_b'):
        V[n] = b.io.tile([C, 1], f32, name=n, tag=n)
        nc.sync.dma_start(out=V[n], in_=p[n][:, :])
    ones_col = b.io.tile([C, 1], bf16, name='ones_col', tag='ones_col')
    nc.vector.memset(ones_col, 1.0)
    b.ones_col = ones_col
    ones_colf = b.io.tile([C, 1], f32, name='ones_colf', tag='ones_colf')
    nc.vector.memset(ones_colf, 1.0)
    ones_row = b.io.tile([1, 128], bf16, name='ones_row', tag='ones_row')
    nc.vector.memset(ones_row, 1.0)
    b.ones_row = ones_row
    eps_t = b.io.tile([1, 1], f32, name='lneps', tag='lneps')
    nc.vector.memset(eps_t, 1e-5)
    b.eps_t = eps_t

    # ---- P1: input layernorm ----
    nrm = b.pb.tile([C, L], bf16, name='nrm', tag='nrmo')
    _ln_stats_mm(b, x, ones_colf, nrm)

    # ---- P2 ----
    xmf = b.pb.tile([C, L], bf16, name='xmf', tag='xmf')
    _proj(b, W['wlmT'], nrm, xmf, AF.Identity, V['bias_lm'][:, :])
    xm_pad = b.pb.tile([C, D_CONV - 1 + L], bf16, name='xm_pad', tag='xm_pad')
    nc.vector.memset(xm_pad[:, 0:D_CONV - 1], 0.0)
    _proj(b, W['wcT'], xmf, xm_pad, AF.Silu, V['bias_c'][:, :],
          out_off=D_CONV - 1)

    uc = []
    for h in range(2):
        uct = b.pb.tile([128, L], bf16, name=f'uc{h}', tag=f'uc{h}')
        for pi in range(NPW):
            psu = b.ps.tile([128, PW], f32, name='bank', tag='bank')
            for half in range(2):
                base = pi * PW + half * CW
                for k in range(D_CONV):
                    nc.tensor.matmul(psu[:, half * CW:(half + 1) * CW],
                                     W[f'wk{h}{k}'],
                                     xm_pad[:, k + base:k + base + CW],
                                     start=(k == 0), stop=(k == D_CONV - 1))
            nc.scalar.activation(uct[:, pi * PW:(pi + 1) * PW], psu, AF.Silu,
                                 bias=V['conv_b'][:, h:h + 1])
        uc.append(uct)

    # dbl rows: 0-15 B, 16-31 C, 32-39 dtr (xp_w rows reordered host-side)
    dbl_sb = b.pb.tile([40, L], bf16, name='dbl_sb', tag='dbl_sb')
    dtr = dbl_sb[32:40, :]
    for pi in range(NPW):
        psd = b.ps.tile([40, PW], f32, name='bank', tag='bank')
        for half in range(2):
            hs = slice(half * CW, (half + 1) * CW)
            cs = slice(pi * PW + half * CW, pi * PW + (half + 1) * CW)
            nc.tensor.matmul(psd[:, hs], W['xpwT0'], uc[0][:, cs],
                             start=True, stop=False)
            nc.tensor.matmul(psd[:, hs], W['xpwT1'], uc[1][:, cs],
                             start=False, stop=True)
        nc.scalar.activation(dbl_sb[:, pi * PW:(pi + 1) * PW], psd,
                             AF.Identity, bias=0.0)
    bc_d = b.dram.tile([32, L], bf16, name='bc_d', tag='bc_d')
    nc.sync.dma_start(out=bc_d, in_=dbl_sb[0:32, :])

    # dt = ln(1 + exp(dt_w @ dtr + dt_b)); f32 copy feeds the da exps,
    # bf16 copy feeds the 2x-mode dtu multiply
    dt = []
    dtu = []
    for h in range(2):
        z1 = b.pf.tile([128, L], f32, name=f'z1{h}', tag='f')
        _proj(b, W[f'dtwT{h}'], dtr, z1, AF.Exp, V['dt_b'][:, h:h + 1],
              rows=128)
        dtt = b.pb.tile([128, L], f32, name=f'dt{h}', tag=f'dt{h}')
        nc.scalar.activation(dtt, z1, AF.Ln, bias=1.0, scale=1.0)
        dtt_bf = b.pb.tile([128, L], bf16, name=f'dtb{h}', tag=f'dtb{h}')
        nc.scalar.activation(dtt_bf, z1, AF.Ln, bias=1.0, scale=1.0)
        dt.append(dtt)
        dtut = b.pb.tile([128, L], bf16, name=f'dtu{h}', tag=f'dtu{h}')
        nc.vector.tensor_tensor(dtut, dtt_bf, uc[h], mult)
        dtu.append(dtut)

    sz = []
    yz = []

    def scan_block(h):
        psy = b.py.tile([128, L], f32, name='psy', tag='psy')
        for s in range(D_STATE):
            j = 16 * h + s
            b_bc = b.bc.tile([128, L], bf16, name='b_bc', tag='b_bc')
            src = bass.AP(tensor=bc_d.tensor, offset=bc_d.offset + s * L,
                          ap=[[0, 128], [1, L]])
            nc.sync.dma_start(out=b_bc, in_=src)
            c_bc = b.cb.tile([128, L], bf16, name='c_bc', tag='c_bc')
            src = bass.AP(tensor=bc_d.tensor, offset=bc_d.offset + (16 + s) * L,
                          ap=[[0, 128], [1, L]])
            nc.sync.dma_start(out=c_bc, in_=src)

            da = b.da.tile([128, L], f32, name='da', tag='da')
            nc.scalar.activation(da, dt[h], AF.Exp, bias=0.0,
                                 scale=V['avec'][:, j:j + 1])
            dbu = b.du.tile([128, L], bf16, name='dbu', tag='dbu')
            nc.vector.tensor_tensor(dbu, dtu[h], b_bc, mult)
            ht = b.ht.tile([128, L], bf16, name='ht', tag='ht')
            nc.vector.tensor_tensor_scan(ht, da, dbu, 0.0, mult, add)
            ycm = b.yc.tile([128, L], bf16, name='ycm', tag='ycm')
            nc.vector.tensor_tensor(ycm, ht, c_bc, mult)
            for ci in range(NCH):
                cs = slice(ci * CW, (ci + 1) * CW)
                nc.tensor.matmul(psy[:, cs], W['ident'], ycm[:, cs],
                                 start=(s == 0), stop=False,
                                 skip_group_check=True)
        # fold uc * D into psy on PE (diagonal weights), closing the group
        for ci in range(NCH):
            cs = slice(ci * CW, (ci + 1) * CW)
            nc.tensor.matmul(psy[:, cs], W[f'diagD{h}'], uc[h][:, cs],
                             start=False, stop=True, skip_group_check=True)
        return psy

    def yz_block(h, psy):
        # reuses dtu[h]'s buffer: dtu[h]'s last read is state 15's dbu
        yq = b.pb.tile([128, L], bf16, name=f'yq{h}', tag=f'dtu{h}')
        nc.scalar.activation(yq, psy, AF.Identity, bias=0.0)
        yzt = b.pb.tile([128, L], bf16, name=f'yz{h}', tag=f'yz{h}')
        nc.vector.tensor_tensor(yzt, yq, sz[h], mult)
        yz.append(yzt)

    def outproj_half(h, y_out):
        """y_out = owTA_h.T @ yz_h + owTB_h.T @ reversed(yz_h): per-core
        masked weights make this the direction-selected oriented output."""
        for pi in range(NPW):
            ps = b.ps.tile([C, PW], f32, name='bank', tag='bank')
            for half in range(2):
                hs = slice(half * CW, (half + 1) * CW)
                a0 = pi * PW + half * CW
                a1 = pi * PW + (half + 1) * CW
                nc.tensor.matmul(ps[:, hs], W[f'owTA{h}'], yz[h][:, a0:a1],
                                 start=True, stop=False)
                rcs = yz[h][:, L - a1:L - a0][:, ::-1]
                nc.tensor.matmul(ps[:, hs], W[f'owTB{h}'], rcs,
                                 start=False, stop=True)
            nc.scalar.activation(y_out[:, pi * PW:(pi + 1) * PW], ps,
                                 AF.Identity, bias=0.0)

    # h = 0 scans; gate/z projections run on ACT/PE meanwhile
    psy0 = scan_block(0)
    gate = b.pb.tile([C, L], bf16, name='gate', tag='gate')
    _proj(b, W['wlgT'], nrm, gate, AF.Silu, V['bias_lg'][:, :])
    for h in range(2):
        szt = b.pb.tile([128, L], bf16, name=f'sz{h}', tag=f'sz{h}')
        _proj(b, W[f'inwzT{h}'],
              xm_pad[:, D_CONV - 1:D_CONV - 1 + L], szt, AF.Silu, 0.0)
        sz.append(szt)
    yz_block(0, psy0)

    # h=0 out-projection + its AllReduce, hidden under the h=1 scan block
    y0 = b.pb.tile([C, L], bf16, name='y0', tag='y0')
    outproj_half(0, y0)
    cc_in0 = b.dram.tile([C, L], bf16, name='cc_in0', tag='cc_in0')
    cc_out0 = b.dram.tile([C, L], bf16, name='cc_out0', tag='cc_out0')
    nc.sync.dma_start(out=cc_in0, in_=y0)
    nc.gpsimd.collective_compute(
        'AllReduce', add,
        replica_groups=[[0, 1], [2, 3], [4, 5], [6, 7]],
        ins=[cc_in0.opt()], outs=[cc_out0.opt()])

    psy1 = scan_block(1)
    yz_block(1, psy1)

    y1t = b.pb.tile([C, L], bf16, name='y1t', tag='y1t')
    outproj_half(1, y1t)
    ysum2 = b.pb.tile([C, L], bf16, name='ysum2', tag='ysum2')
    cc_in1 = b.dram.tile([C, L], bf16, name='cc_in1', tag='cc_in1')
    cc_out1 = b.dram.tile([C, L], bf16, name='cc_out1', tag='cc_out1')
    nc.sync.dma_start(out=cc_in1, in_=y1t)
    nc.gpsimd.collective_compute(
        'AllReduce', add,
        replica_groups=[[0, 1], [2, 3], [4, 5], [6, 7]],
        ins=[cc_in1.opt()], outs=[cc_out1.opt()])

    # ---- P5, split so the h0 part runs in AR1's latency shadow ----
    y_sum = b.pb.tile([C, L], bf16, name='y_sum', tag='y0')
    nc.sync.dma_start(out=y_sum, in_=cc_out0)
    g1a = b.pb.tile([C, L], bf16, name='g1a', tag='nrmo')
    nc.vector.tensor_tensor(g1a, y_sum, gate, mult)
    ps_t2 = []
    for pi in range(NPW):
        ps = b.ps.tile([C, PW], f32, name='bank', tag='bank')
        for half in range(2):
            cs = slice(pi * PW + half * CW, pi * PW + (half + 1) * CW)
            nc.tensor.matmul(ps[:, half * CW:(half + 1) * CW], W['loT'],
                             g1a[:, cs], start=True, stop=False,
                             skip_group_check=True)
        ps_t2.append(ps)
    nc.sync.dma_start(out=ysum2, in_=cc_out1)
    g1b = b.pb.tile([C, L], bf16, name='g1b', tag='y1t')
    nc.vector.tensor_tensor(g1b, ysum2, gate, mult)
    t2 = b.pb.tile([C, L], bf16, name='t2', tag='t2')
    for pi in range(NPW):
        for half in range(2):
            cs = slice(pi * PW + half * CW, pi * PW + (half + 1) * CW)
            nc.tensor.matmul(ps_t2[pi][:, half * CW:(half + 1) * CW], W['loT'],
                             g1b[:, cs], start=False, stop=True,
                             skip_group_check=True)
        ocs = slice(pi * PW, (pi + 1) * PW)
        nc.scalar.activation(t2[:, ocs], ps_t2[pi], AF.Identity,
                             bias=V['lo_b'][:, :])

    o1 = b.pb.tile([C, L], bf16, name='o1', tag='xm_pad')
    _ln_stats_mm(b, t2, b.ones_col, o1)
    out_sb = b.pf.tile([C, L], f32, name='out_sb', tag='f')
    nc.scalar.activation(out_sb, o1, AF.Identity, bias=V['ln_b'][:, :],
                         scale=V['ln_g'][:, :])
    nc.sync.dma_start(out=p['y'][:, :], in_=out_sb)


def _build_program():
    import contextlib
    nc = bacc.Bacc('TRN2', target_bir_lowering=False, debug=False, num_devices=8)
    p = _declare(nc)
    with tile.TileContext(nc) as tc:
        with contextlib.ExitStack() as ctx:
            _build_body(nc, tc, p, ctx)
    nc.compile()
    return nc


def _prep_core_inputs(inputs, bidx, d):
    g = lambda n: np.asarray(inputs[n], dtype=np.float32)
    x = g('x')
    ln_g = g('ln_g')
    ln_b = g('ln_b')
    pre = 'mf_' if d == 0 else 'mb_'
    P = lambda n: np.asarray(inputs[pre + n], dtype=np.float32)

    lm_w, lm_b = g('lm_w'), g('lm_b')
    lg_w, lg_b = g('lg_w'), g('lg_b')
    lo_w, lo_b = g('lo_w'), g('lo_b')
    if d == 0:
        wc, cb = g('cf_w'), g('cf_b')
    else:
        wc, cb = np.ascontiguousarray(g('cb_w')[:, ::-1]), g('cb_b')

    A = -np.exp(P('Alog'))
    avec = np.zeros((128, 32), np.float32)
    for h in range(2):
        for s in range(16):
            avec[:, 16 * h + s] = A[128 * h:128 * (h + 1), s]

    bf = lambda a: np.ascontiguousarray(np.asarray(a, dtype=ml_dtypes.bfloat16))
    col = lambda v: np.ascontiguousarray(v.astype(np.float32).reshape(-1, 1))
    halves = lambda v: np.ascontiguousarray(
        np.stack([v[:128], v[128:]], axis=1).astype(np.float32))
    T = lambda w: np.ascontiguousarray(w.T)

    in_w = P('in_w')
    conv_w = P('conv_w')
    xpw = P('xp_w')
    xpw = np.concatenate([xpw[DT_RANK:], xpw[:DT_RANK]], axis=0)
    xpwT = np.ascontiguousarray(xpw.T)
    outwT = np.ascontiguousarray(P('out_w').T)
    dtwT = np.ascontiguousarray(P('dt_w').T)

    out = {
        'x': np.ascontiguousarray(x[bidx]),
        'wlmT': bf(T(lm_w * ln_g[None, :])),
        'wlgT': bf(T(lg_w * ln_g[None, :])),
        'wcT': bf(T(wc)),
        'loT': bf(T(lo_w)),
        'ident': bf(np.eye(128, dtype=np.float32)),
        'avec': avec,
        'conv_b': halves(P('conv_b')),
        'dt_b': halves(P('dt_b')),
        'bias_lm': col(lm_w @ ln_b + lm_b),
        'bias_lg': col(lg_w @ ln_b + lg_b),
        'bias_c': col(cb),
        'lo_b': col(lo_b),
        'ln_g': col(ln_g),
        'ln_b': col(ln_b),
    }
    for h in range(2):
        hsl = slice(128 * h, 128 * (h + 1))
        out[f'diagD{h}'] = bf(np.diag(P('D')[hsl]).astype(np.float32))
        for k in range(D_CONV):
            wk = in_w[hsl, :] * conv_w[hsl, k:k + 1]
            out[f'wk{h}{k}'] = bf(T(wk))
        out[f'inwzT{h}'] = bf(T(P('in_w')[256:][hsl, :]))
        ow = outwT[hsl, :]
        out[f'owTA{h}'] = bf(ow if d == 0 else np.zeros_like(ow))
        out[f'owTB{h}'] = bf(np.zeros_like(ow) if d == 0 else ow)
        out[f'xpwT{h}'] = bf(xpwT[hsl, :])
        out[f'dtwT{h}'] = bf(dtwT[:, hsl])
    return out


def get_program():
    global _PROGRAM
    if _PROGRAM is None:
        _PROGRAM = _build_program()
    return _PROGRAM


def run(inputs, **run_kwargs):
    nc = get_program()
    in_maps = [_prep_core_inputs(inputs, c // 2, c % 2) for c in range(8)]
    res = run_bass_kernel_spmd(nc, in_maps, core_ids=list(range(8)), **run_kwargs)
    out = np.stack([res.results[2 * b]['y'] for b in range(BATCH)], axis=0)
    return out, res


def kernel(**inputs) -> np.ndarray:
    out, _ = run(inputs)
    return out.astype(np.float32)

